# revision 21
# baseline (speedup 1.0000x reference)
"""GCN+JumpingKnowledge inference kernel for Trainium2 (8 NeuronCores). v2.

Computation (PyG GCNConv defaults, eval mode):
    deg[v]  = in_degree(v) + 1  (self loops)
    dis     = deg ** -0.5
    agg(x)[v] = sum over edges (u->v) incl self loop of dis[u]*dis[v]*x[u]
    x1 = relu(BN(agg(node_feat) @ W1 + b1))      (BN folded into W1A/D1)
    x2 = agg(x1 @ W2 scaled) + b2
    out = max(x1, x2) @ Wf + bf

v2 changes vs v1:
  * No dense phase A: layer-1 messages are gathered straight from the
    host-supplied row-major table nfr = dis_u * node_feat (the W1A
    transform is applied AFTER aggregation, once per 128-dest group).
  * The h2 AllGather is split into `pieces` collectives, each issued as
    soon as its groups' epilogues are done, overlapping the collective
    with the tail of layer 1 and the head of layer 2.
  * Layer-2 gather chunks follow the AG piece tables.

Aggregation (both layers): edges sorted by (dest-group, source-chunk);
source rows fetched with SWDGE dma_gather (int16 idx); per 128-edge tile
a selection matrix S[e,d] = (dloc[e] == d) is built on DVE and the
segment sum is S.T @ msg on the PE into one PSUM tile per dest group.
Self loops are appended to the edge list.
"""

import math
import os
from contextlib import ExitStack

import numpy as np

# ---------------- problem constants (hardcoded by contract) ----------------
N = 100000
E = 1600000
FIN = 128
HID = 128
FOUT = 40
BN_EPS = 1e-5
NCORES = 8


def _chunk_ranges(total, chunk):
    """Split [0, total) into ranges of <= chunk rows; the final range is
    [total-chunk, total) so every index stays in [0, chunk)."""
    starts, bases = [], []
    s = 0
    while True:
        starts.append(s)
        if total - s <= chunk:
            bases.append(max(0, min(s, total - chunk)))
            break
        bases.append(s)
        s += chunk
    rows = [min(chunk, total - b) for b in bases]
    return starts, bases, rows


class Config:
    def __init__(self, n=N, ncores=NCORES, groups_per_core=98, wave=6,
                 chunk=32768, msg_bf16=True, pieces=4, gmax=1024, queues=4,
                 scratch=16384, msg_bufs=8, agg_bufs=None, ep_bufs=2,
                 sbuild="tt"):
        self.n = n
        self.ncores = ncores
        self.G = groups_per_core
        self.shard = 128 * groups_per_core
        self.npad = self.shard * ncores
        self.wave = wave
        self.chunk = chunk
        self.msg_bf16 = msg_bf16
        self.gmax = gmax
        self.queues = queues
        self.scratch = scratch
        self.msg_bufs = msg_bufs
        self.agg_bufs = agg_bufs if agg_bufs is not None else wave
        self.ep_bufs = ep_bufs
        self.sbuild = sbuild
        # --- layer-1 chunks: int16 ranges over the full node table ---
        self.l1_starts, self.l1_bases, self.l1_rows = _chunk_ranges(
            self.npad, chunk)
        # --- AG pieces: wave-aligned group ranges, waves split evenly ---
        nw = -(-self.G // wave)
        wsplit = [nw // pieces + (1 if p < nw % pieces else 0)
                  for p in range(pieces)]
        pg = [0]
        for p in range(pieces):
            pg.append(min(self.G, pg[-1] + wsplit[p] * wave))
        self.pg = pg                       # group boundaries per piece
        self.pieces = len(pg) - 1
        self.piece_rows = [(pg[j + 1] - pg[j]) * 128
                           for j in range(self.pieces)]
        # piece j's gathered table has ncores * piece_rows[j] rows; split
        # into int16 chunks.  l2 chunk list: (piece, start, base, rows)
        self.l2_chunks = []
        for j in range(self.pieces):
            tot = ncores * self.piece_rows[j]
            starts, bases, rows = _chunk_ranges(tot, chunk)
            for s, b, r in zip(starts, bases, rows):
                self.l2_chunks.append((j, s, b, r))
        # wave index after which AG piece j fires
        self.piece_wave = [
            max(0, -(-pg[j + 1] // wave) - 1) for j in range(self.pieces)]

    def l1_chunk_of(self, u):
        k = np.minimum(np.searchsorted(self.l1_starts, u, side="right") - 1,
                       len(self.l1_starts) - 1)
        return k, u - np.asarray(self.l1_bases, np.int64)[k]

    def l2_chunk_of(self, u):
        """Map global source node -> (l2 chunk id, local row)."""
        rank = u // self.shard
        off = u - rank * self.shard
        gg = off >> 7
        pg = np.asarray(self.pg[1:-1], np.int64)
        j = np.searchsorted(pg, gg, side="right")
        prows = np.asarray(self.piece_rows, np.int64)[j]
        pg0 = np.asarray(self.pg, np.int64)[j]
        row = rank * prows + (off - pg0 * 128)
        # chunk within piece
        piece_first_chunk = {}
        ck = np.zeros_like(u)
        lidx = np.zeros_like(u)
        for ci, (pj, s, b, r) in enumerate(self.l2_chunks):
            m = (j == pj) & (row >= s) & ((row - s < self.chunk) if True
                                          else True)
            # assign by range: row in [s, next_start)
            piece_first_chunk.setdefault(pj, ci)
        # simpler: per piece, searchsorted over that piece's starts
        for pj in range(self.pieces):
            cs = [(ci, s, b) for ci, (p2, s, b, r) in
                  enumerate(self.l2_chunks) if p2 == pj]
            starts = np.asarray([s for _, s, _ in cs], np.int64)
            bases = np.asarray([b for _, _, b in cs], np.int64)
            cids = np.asarray([ci for ci, _, _ in cs], np.int64)
            m = (j == pj)
            loc = np.minimum(
                np.searchsorted(starts, row[m], side="right") - 1,
                len(cs) - 1)
            ck[m] = cids[loc]
            lidx[m] = row[m] - bases[loc]
        return ck, lidx


CFG = Config(sbuild="ts")


# ---------------------------- host preprocessing ---------------------------

class Sched:
    """Program schedule shared by every core (max over per-core needs)."""

    def __init__(self, G, wave, M):
        self.G = G
        self.K = M.shape[1]
        self.M = M
        self.waves = [list(range(w, min(w + wave, G)))
                      for w in range(0, G, wave)]
        self.segs = []      # (w, k, slot_off, n_slots, subtiles)
        self.run_bounds = {}
        self.tot_slots = 0
        self.tot_tiles = 0
        grp_subtiles = {g: [] for g in range(G)}
        for w, groups in enumerate(self.waves):
            for k in range(self.K):
                seg_used = int(M[groups, k].sum())
                if seg_used == 0:
                    continue
                n_slots = -(-seg_used // 128) * 128
                off = self.tot_slots
                pos = off
                for g in groups:
                    if M[g, k]:
                        self.run_bounds[(g, k)] = (pos, pos + int(M[g, k]))
                        pos += int(M[g, k])
                subtiles = []
                for lt in range(n_slots // 128):
                    t_lo, t_hi = off + lt * 128, off + (lt + 1) * 128
                    for g in groups:
                        b = self.run_bounds.get((g, k))
                        if b and b[0] < t_hi and b[1] > t_lo:
                            subtiles.append([lt, g, False, False])
                            grp_subtiles[g].append(subtiles[-1])
                self.segs.append((w, k, off, n_slots, subtiles))
                self.tot_slots += n_slots
                self.tot_tiles += len(subtiles)
        for g in range(G):
            sts = grp_subtiles[g]
            assert sts, f"group {g} has no subtiles"
            sts[0][2] = True
            sts[-1][3] = True


def _bucket(cfg, row, col, core, chunk_of):
    """Per-core edge stream (with self loops) keyed by (group, chunk)."""
    lo, hi = core * cfg.shard, (core + 1) * cfg.shard
    m = (col >= lo) & (col < hi)
    r, c = row[m], col[m]
    g = (c - lo) >> 7
    k, lidx = chunk_of(r)
    dloc = (c - lo) & 127
    order = np.lexsort((k, g))
    return g[order], k[order], lidx[order], dloc[order]


def _layer_sched(cfg, row, col, chunk_of, nchunks):
    per_core = [_bucket(cfg, row, col, c, chunk_of)
                for c in range(cfg.ncores)]
    counts = np.zeros((cfg.ncores, cfg.G, nchunks), np.int64)
    for c, (g, k, _, _) in enumerate(per_core):
        np.add.at(counts[c], (g, k), 1)
    M = counts.max(axis=0)
    sched = Sched(cfg.G, cfg.wave, M)

    slot_group = np.full(sched.tot_slots, -1, np.int64)
    for (g, k), (lo, hi) in sched.run_bounds.items():
        slot_group[lo:hi] = g

    idx_all, dloc_all = [], []
    for c in range(cfg.ncores):
        g, k, lidx, dloc = per_core[c]
        slots_idx = np.zeros(sched.tot_slots, np.int32)
        slots_dloc = np.full(sched.tot_slots, -1.0, np.float32)
        key = g * nchunks + k
        bounds = np.searchsorted(key, np.arange(cfg.G * nchunks + 1))
        for (gg, kk), (lo, hi) in sched.run_bounds.items():
            b0, b1 = bounds[gg * nchunks + kk], bounds[gg * nchunks + kk + 1]
            nrun = b1 - b0
            assert nrun <= hi - lo
            slots_idx[lo:lo + nrun] = lidx[b0:b1]
            slots_dloc[lo:lo + nrun] = dloc[b0:b1]
        idx_w = np.zeros((128, sched.tot_slots // 16), np.int16)
        for w, kk, seg_off, n_slots, subtiles in sched.segs:
            seg = slots_idx[seg_off:seg_off + n_slots]
            wrapped = seg.reshape(-1, 16).T.astype(np.int16)
            idx_w[:, seg_off // 16:(seg_off + n_slots) // 16] = np.tile(
                wrapped, (8, 1))
        dloc_w = np.full((128, sched.tot_tiles), -1.0, np.float32)
        tid = 0
        for w, kk, seg_off, n_slots, subtiles in sched.segs:
            for lt, gg, first, last in subtiles:
                t_lo = seg_off + lt * 128
                sl = slice(t_lo, t_lo + 128)
                dloc_w[:, tid] = np.where(slot_group[sl] == gg,
                                          slots_dloc[sl], -1.0)
                tid += 1
        idx_all.append(idx_w)
        dloc_all.append(np.ascontiguousarray(dloc_w))
    return sched, idx_all, dloc_all


def prepare(cfg, edge_index):
    row = np.asarray(edge_index[0], np.int64)
    col = np.asarray(edge_index[1], np.int64)
    deg = np.bincount(col, minlength=cfg.n).astype(np.float32) + 1.0
    dis = 1.0 / np.sqrt(deg)
    dis_pad = np.zeros(cfg.npad, np.float32)
    dis_pad[:cfg.n] = dis

    s1, idx1, dloc1 = _layer_sched(cfg, row, col, cfg.l1_chunk_of,
                                   len(cfg.l1_starts))
    s2, idx2, dloc2 = _layer_sched(cfg, row, col, cfg.l2_chunk_of,
                                   len(cfg.l2_chunks))
    return (s1, s2), dis_pad, (idx1, idx2), (dloc1, dloc2)


# ------------------------------- bass builder ------------------------------

def build_module(cfg, scheds, reps=1, single=False, mode="full"):
    import concourse.bacc as bacc
    import concourse.tile as tile
    from concourse import mybir
    import concourse.bass as bass

    s1, s2 = scheds
    f32 = mybir.dt.float32
    mdt = mybir.dt.bfloat16 if cfg.msg_bf16 else f32
    i16 = mybir.dt.int16
    eq = mybir.AluOpType.is_equal
    mult = mybir.AluOpType.mult
    add = mybir.AluOpType.add
    amax = mybir.AluOpType.max
    Act = mybir.ActivationFunctionType

    nc = bacc.Bacc("TRN2", target_bir_lowering=False, debug=False,
                   num_devices=1 if single else cfg.ncores,
                   num_swdge_queues=cfg.queues,
                   dynamic_dma_scratch_size=cfg.scratch)
    npad, shard, G = cfg.npad, cfg.shard, cfg.G

    ein, eout = "ExternalInput", "ExternalOutput"
    nfr_d = nc.dram_tensor("nfr", [npad, FIN], mdt, kind=ein)
    own1_d = nc.dram_tensor("own1", [shard, FIN], mdt, kind=ein)
    idx1_d = nc.dram_tensor("idx1", [128, s1.tot_slots // 16], i16, kind=ein)
    idx2_d = nc.dram_tensor("idx2", [128, s2.tot_slots // 16], i16, kind=ein)
    ddt = f32 if cfg.sbuild == "ts" else mdt
    dloc1_d = nc.dram_tensor("dloc1", [128, s1.tot_tiles], ddt, kind=ein)
    dloc2_d = nc.dram_tensor("dloc2", [128, s2.tot_tiles], ddt, kind=ein)
    diss_d = nc.dram_tensor("diss", [128, G], f32, kind=ein)
    w1a_d = nc.dram_tensor("w1a", [FIN, HID], mdt, kind=ein)
    w2_d = nc.dram_tensor("w2", [HID, HID], mdt, kind=ein)
    wf_d = nc.dram_tensor("wf", [HID, FOUT], mdt, kind=ein)
    d1_d = nc.dram_tensor("d1t", [128, HID], f32, kind=ein)
    b2_d = nc.dram_tensor("b2t", [128, HID], f32, kind=ein)
    bf_d = nc.dram_tensor("bft", [128, FOUT], f32, kind=ein)
    iota_d = nc.dram_tensor("iota", [128, 128], mdt, kind=ein)
    identb_d = nc.dram_tensor("identb", [128, 128], mdt, kind=ein)
    out_d = nc.dram_tensor("out", [shard, FOUT], f32, kind=eout)
    h2s_d = nc.dram_tensor("h2shard", [shard, HID], mdt)
    h2p_d = [nc.dram_tensor(f"h2p{j}", [cfg.ncores * cfg.piece_rows[j], HID],
                            mdt, addr_space="Shared")
             for j in range(cfg.pieces)]

    with tile.TileContext(nc) as tc, ExitStack() as ctx:
        from concourse.library_config import mlp as mlp_lib
        nc.gpsimd.load_library(mlp_lib)

        consts = ctx.enter_context(tc.tile_pool(name="consts", bufs=1))
        aggp = ctx.enter_context(tc.tile_pool(name="aggp",
                                              bufs=cfg.agg_bufs,
                                              space="PSUM"))
        psum = ctx.enter_context(tc.tile_pool(name="psum",
                                              bufs=cfg.ep_bufs,
                                              space="PSUM"))
        idxp = ctx.enter_context(tc.tile_pool(name="idx", bufs=4))
        dlp = ctx.enter_context(tc.tile_pool(name="dl", bufs=4))
        msgp = ctx.enter_context(tc.tile_pool(name="msg",
                                              bufs=cfg.msg_bufs))
        spool = ctx.enter_context(tc.tile_pool(name="s", bufs=4))
        x1pool = ctx.enter_context(tc.tile_pool(name="x1", bufs=G))
        epool = ctx.enter_context(tc.tile_pool(name="ep", bufs=6))

        def load_const(dram, shape, dtype):
            t = consts.tile(shape, dtype, tag=dram.name)
            nc.sync.dma_start(t[:], dram.ap())
            return t

        w1a = load_const(w1a_d, [FIN, HID], mdt)
        w2 = load_const(w2_d, [HID, HID], mdt)
        wf = load_const(wf_d, [HID, FOUT], mdt)
        d1t = load_const(d1_d, [128, HID], f32)
        b2t = load_const(b2_d, [128, HID], f32)
        bft = load_const(bf_d, [128, FOUT], f32)
        iota = load_const(iota_d, [128, 128], mdt)
        identb = load_const(identb_d, [128, 128], mdt)
        diss = load_const(diss_d, [128, G], f32)


        qctr = [0]

        def edge_phase(rep, lname, sched, idx_d, dloc_d, tables, epilogue,
                       self_table, after_wave=None):
            """tables: list over chunk k of (dram_tensor, base, rows)."""
            do_gather = mode in ("full", "l1", "gather1", "l2", "l2g")
            do_compute = mode in ("full", "l1", "sb1", "l2") or \
                (mode == "l2g" and lname == "never")
            do_epi = mode in ("full", "l1", "l2")
            tid0 = [0]
            for w, groups in enumerate(sched.waves):
                gps = {g: aggp.tile([128, HID], f32, tag="agg",
                                    name=f"agg_{rep}_{lname}_w{w}_g{g}")[:]
                       for g in groups}
                if do_compute:
                    # self-loop term: psum[d] starts at self_table[d] (the
                    # dis_u-scaled own row; dis_d applied in the epilogue)
                    ow = msgp.tile([128, len(groups), FIN], mdt, tag="own")
                    g0 = groups[0]
                    nc.sync.dma_start(
                        ow[:, :len(groups), :],
                        self_table[g0 * 128:(g0 + len(groups)) * 128, :]
                        .rearrange("(j p) f -> p j f", p=128))
                    for gi, g in enumerate(groups):
                        nc.tensor.matmul(gps[g], lhsT=identb[:],
                                         rhs=ow[:, gi, :],
                                         start=True, stop=False)
                wave_segs = [s for s in sched.segs if s[0] == w]
                w_off = wave_segs[0][2]
                w_slots = sum(s[3] for s in wave_segs)
                idx_sb = idxp.tile([128, w_slots // 16], i16, tag="idx")
                if do_gather:
                    nc.sync.dma_start(
                        idx_sb[:],
                        idx_d[:, w_off // 16:(w_off + w_slots) // 16])
                w_tiles = sum(len(s[4]) for s in wave_segs)
                wt0 = tid0[0]
                if do_compute:
                    dloc = dlp.tile([128, w_tiles], ddt,
                                    tag="dl" + lname)
                    nc.sync.dma_start(dloc[:],
                                      dloc_d[:, wt0:wt0 + w_tiles])
                for (sw, k, seg_off, n_slots, subtiles) in wave_segs:
                    so16 = (seg_off - w_off) // 16
                    J = n_slots // 128
                    msg = msgp.tile([128, J, 128], mdt, tag="msg")
                    tab_d, base, rows = tables[k]
                    for sub in range(0, n_slots, cfg.gmax) if do_gather \
                            else []:
                        ns = min(cfg.gmax, n_slots - sub)
                        nc.gpsimd.dma_gather(
                            out_ap=msg[:, sub // 128:(sub + ns) // 128, :],
                            in_ap=tab_d[base:base + rows, :],
                            idxs_ap=idx_sb[:, so16 + sub // 16:
                                           so16 + (sub + ns) // 16],
                            num_idxs=ns,
                            num_idxs_reg=ns,
                            elem_size=HID,
                            queue_num=qctr[0] % cfg.queues,
                        )
                        qctr[0] += 1
                    if not do_compute:
                        continue
                    nst = len(subtiles)
                    sb = spool.tile([128, nst, 128], mdt, tag="s",
                                    name=f"s_{rep}_{lname}_w{w}_k{k}")
                    lt0 = tid0[0] - wt0
                    if cfg.sbuild == "tt":
                        h0 = (nst + 1) // 2
                        for lo, hi in ((0, h0), (h0, nst)):
                            if hi == lo:
                                continue
                            nb = hi - lo
                            io_b = iota[:].unsqueeze(1).broadcast_to(
                                [128, nb, 128])
                            dl_b = (dloc[:, lt0 + lo:lt0 + hi]
                                    .unsqueeze(2)
                                    .broadcast_to([128, nb, 128]))
                            nc.vector.tensor_tensor(sb[:, lo:hi, :],
                                                    io_b, dl_b, op=eq)
                    else:
                        for sj in range(nst):
                            nc.vector.tensor_scalar(
                                sb[:, sj, :], iota[:],
                                dloc[:, lt0 + sj:lt0 + sj + 1],
                                None, op0=eq)
                    tid0[0] += nst
                    for sj, (lt, g, first, last) in enumerate(subtiles):
                        nc.tensor.matmul(gps[g], lhsT=sb[:, sj, :],
                                         rhs=msg[:, lt, :],
                                         start=False, stop=last)
                if do_epi:
                    for g in groups:
                        epilogue(g, gps[g])
                if after_wave is not None:
                    after_wave(w)

        for rep in range(reps):
            x1_tiles = [x1pool.tile([128, HID], mdt, tag="x1",
                                    name=f"x1_{rep}_{g}")
                        for g in range(G)]

            # ---------------- layer 1 ------------------------------------
            def epilogue1(g, ps):
                # ps[d, fin] = sum_e dis_u * nf[u]; conv1 = dis_d*ps @ W1A
                sc = epool.tile([128, 128], mdt, tag="sc")
                nc.scalar.activation(sc[:], ps, Act.Copy,
                                     scale=diss[:, g:g + 1])
                pt = psum.tile([128, 128], mdt, tag="ps", name=f"pt1_{g}")
                nc.tensor.transpose(pt[:], sc[:], identb[:])
                st = epool.tile([128, 128], mdt, tag="st")
                nc.scalar.activation(st[:], pt[:], Act.Copy)
                p2 = psum.tile([128, HID], f32, tag="ps", name=f"p2_{g}")
                nc.tensor.matmul(p2[:], lhsT=st[:], rhs=w1a[:], start=True,
                                 stop=True)
                x1 = x1_tiles[g]
                nc.vector.tensor_tensor(x1[:], p2[:], d1t[:], op=add)
                nc.scalar.activation(x1[:], x1[:], Act.Relu)
                # h2' = dis_d * (x1 @ W2)
                px = psum.tile([128, 128], mdt, tag="ps", name=f"px1_{g}")
                nc.tensor.transpose(px[:], x1[:], identb[:])
                x1t = epool.tile([128, 128], mdt, tag="x1t")
                nc.scalar.activation(x1t[:], px[:], Act.Copy)
                ph = psum.tile([128, HID], f32, tag="ps", name=f"ph_{g}")
                nc.tensor.matmul(ph[:], lhsT=x1t[:], rhs=w2[:], start=True,
                                 stop=True)
                h2 = epool.tile([128, HID], mdt, tag="h2")
                nc.scalar.activation(h2[:], ph[:], Act.Copy,
                                     scale=diss[:, g:g + 1])
                nc.sync.dma_start(h2s_d[g * 128:(g + 1) * 128, :], h2[:])

            issued = [False] * cfg.pieces

            def after_wave1(w):
                if mode != "full":
                    return
                for j in range(cfg.pieces):
                    if not issued[j] and w >= cfg.piece_wave[j]:
                        issued[j] = True
                        lo, hi = cfg.pg[j] * 128, cfg.pg[j + 1] * 128
                        if single:
                            nc.sync.dma_start(
                                h2p_d[j][0:hi - lo, :],
                                h2s_d[lo:hi, :])
                        else:
                            nc.gpsimd.collective_compute(
                                "AllGather",
                                mybir.AluOpType.bypass,
                                replica_groups=[list(range(cfg.ncores))],
                                ins=[h2s_d[lo:hi, :]],
                                outs=[h2p_d[j].ap()],
                            )

            tables1 = [(nfr_d, cfg.l1_bases[k], cfg.l1_rows[k])
                       for k in range(len(cfg.l1_starts))]
            if mode in ("l2", "l2g"):
                for g in range(G):
                    nc.vector.memset(x1_tiles[g][:], 0.0)
                for j in range(cfg.pieces):
                    lo, hi = cfg.pg[j] * 128, cfg.pg[j + 1] * 128
                    nc.gpsimd.collective_compute(
                        "AllGather", mybir.AluOpType.bypass,
                        replica_groups=[list(range(cfg.ncores))],
                        ins=[h2s_d[lo:hi, :]],
                        outs=[h2p_d[j].ap()],
                    )
            else:
                edge_phase(rep, "l1", s1, idx1_d, dloc1_d, tables1,
                           epilogue1, own1_d, after_wave1)

            # ---------------- layer 2 + JK + final ------------------------
            def epilogue2(g, ps):
                x2 = epool.tile([128, HID], f32, tag="x2")
                nc.scalar.activation(x2[:], ps, Act.Copy,
                                     scale=diss[:, g:g + 1])
                nc.vector.tensor_tensor(x2[:], x2[:], b2t[:], op=add)
                jk = epool.tile([128, 128], mdt, tag="jk")
                nc.vector.tensor_tensor(jk[:], x2[:], x1_tiles[g][:],
                                        op=amax)
                px = psum.tile([128, 128], mdt, tag="ps", name=f"px2_{g}")
                nc.tensor.transpose(px[:], jk[:], identb[:])
                xt = epool.tile([128, 128], mdt, tag="xt")
                nc.scalar.activation(xt[:], px[:], Act.Copy)
                po = psum.tile([128, FOUT], f32, tag="ps", name=f"po_{g}")
                nc.tensor.matmul(po[:], lhsT=xt[:], rhs=wf[:], start=True,
                                 stop=True)
                ob = epool.tile([128, FOUT], f32, tag="ob")
                nc.vector.tensor_tensor(ob[:], po[:], bft[:], op=add)
                nc.sync.dma_start(out_d[g * 128:(g + 1) * 128, :], ob[:])

            if mode in ("full", "l2", "l2g"):
                tables2 = [(h2p_d[pj], b, r)
                           for (pj, s, b, r) in cfg.l2_chunks]
                edge_phase(rep, "l2", s2, idx2_d, dloc2_d, tables2,
                           epilogue2, h2s_d)

    nc.compile()
    return nc


# ------------------------------- host driver -------------------------------

def make_in_maps(cfg, scheds, inputs, dis_pad, idx_all, dloc_all):
    node_feat = np.asarray(inputs["node_feat"], np.float32)
    W1 = np.asarray(inputs["W1"], np.float32)
    b1 = np.asarray(inputs["b1"], np.float32)
    gamma1 = np.asarray(inputs["gamma1"], np.float32)
    beta1 = np.asarray(inputs["beta1"], np.float32)
    mean1 = np.asarray(inputs["mean1"], np.float32)
    var1 = np.asarray(inputs["var1"], np.float32)
    W2 = np.asarray(inputs["W2"], np.float32)
    b2 = np.asarray(inputs["b2"], np.float32)
    Wf = np.asarray(inputs["Wf"], np.float32)
    bf = np.asarray(inputs["bf"], np.float32)

    A = gamma1 / np.sqrt(var1 + BN_EPS)
    W1A = (W1 * A[None, :]).astype(np.float32)
    D1 = (b1 * A + beta1 - mean1 * A).astype(np.float32)

    npad = cfg.npad
    nf_pad = np.zeros((npad, FIN), np.float32)
    nf_pad[:cfg.n] = node_feat
    nf_pad *= dis_pad[:, None]
    iota = np.tile(np.arange(128, dtype=np.float32), (128, 1))
    if cfg.msg_bf16:
        import ml_dtypes
        mdt_np = ml_dtypes.bfloat16
    else:
        mdt_np = np.float32
    ddt_np = np.float32 if cfg.sbuild == "ts" else mdt_np

    common = {
        "nfr": nf_pad.astype(mdt_np),
        "w1a": W1A.astype(mdt_np),
        "w2": W2.astype(mdt_np),
        "wf": Wf.astype(mdt_np),
        "d1t": np.tile(D1, (128, 1)),
        "b2t": np.tile(b2, (128, 1)),
        "bft": np.tile(bf, (128, 1)),
        "iota": iota.astype(mdt_np),
        "identb": np.eye(128, dtype=np.float32).astype(mdt_np),
    }
    idx1, idx2 = idx_all
    dloc1, dloc2 = dloc_all
    nfr_mdt = nf_pad.astype(mdt_np)
    in_maps = []
    for c in range(cfg.ncores):
        lo = c * cfg.shard
        diss = dis_pad[lo:lo + cfg.shard].reshape(-1, 128).T
        in_maps.append(dict(
            common,
            own1=np.ascontiguousarray(nfr_mdt[lo:lo + cfg.shard]),
            idx1=idx1[c],
            idx2=idx2[c],
            dloc1=dloc1[c].astype(ddt_np),
            dloc2=dloc2[c].astype(ddt_np),
            diss=np.ascontiguousarray(diss),
        ))
    return in_maps


def run(cfg, inputs, verbose=False):
    import time
    from concourse.bass_utils import run_bass_kernel_spmd
    from concourse.bass_interp import get_hw_module

    t0 = time.time()
    scheds, dis_pad, idx_all, dloc_all = prepare(cfg, inputs["edge_index"])
    if verbose:
        print(f"[prep {time.time()-t0:.1f}s]", flush=True)
    t0 = time.time()
    nc = build_module(cfg, scheds)
    if verbose:
        print(f"[build+compile {time.time()-t0:.1f}s]", flush=True)
    in_maps = make_in_maps(cfg, scheds, inputs, dis_pad, idx_all, dloc_all)
    nc.m = get_hw_module(nc.m)
    res = run_bass_kernel_spmd(nc, in_maps, core_ids=list(range(cfg.ncores)),
                               trace=False)
    out = np.concatenate([r["out"] for r in res.results], axis=0)[:cfg.n]
    return np.asarray(out, np.float32), res


def kernel(**inputs) -> np.ndarray:
    out, _ = run(CFG, inputs)
    return out
